# revision 62
# baseline (speedup 1.0000x reference)
"""Trainium2 Bass kernel for a dense transformer encoder layer.

Reference computation (B=4, N=2048, C=512, H=8 heads, HIDE=2048):
    attn = MHA(q, k, v)                      # full bidirectional softmax attention
    q2   = LN1(q + attn)
    mlp  = fc2(gelu(fc1(q2)))
    out  = LN2(q2 + mlp)

Sharding: data-parallel over (batch x query-sequence-half): 8 cores, each
handles 1024 query rows of one batch, with the full 2048 keys/values of that
batch resident per core.  Zero collectives.

Everything on-device runs in "transposed land" ([C, nq] layouts, channels on
partitions) so that no on-device transposes are needed anywhere:
  * scores_T[nk, nq] = kT_h.T @ qT_h           (lhsT = kT slice, K = 64)
  * probs_T = exp(scores_T / 8)                (ACT, no max-subtraction; scores
                                                are O(6) so exp is safe in fp32)
  * attn_T[d, nq]   = vaug_h.T @ probs_T       (lhsT = v augmented with a ones
                                                column -> row 64 = softmax denom)
  * LN stats over C via ones-column matmuls (partition-axis reduction on PE)
  * rstd = exp(-0.5 * ln(var + eps))           (ACT Rsqrt is banned for accuracy)
  * MLP stays transposed: h_T = fc1_w @ q2_T, mlp_T = fc2_w @ h_T
The final output is written transposed ([C, nq]) and un-transposed on host.
"""

import numpy as np
from contextlib import ExitStack

import concourse.bass as bass
import concourse.mybir as mybir
import concourse.tile as tile
from concourse import bacc
from concourse.bass_utils import run_bass_kernel_spmd

F32 = mybir.dt.float32
F32R = mybir.dt.float32r
AF = mybir.ActivationFunctionType
ALU = mybir.AluOpType


BF16 = mybir.dt.bfloat16

B = 4
H = 8
C = 512
D = 64
HID = 2048
NSEQ = 2048
NQ = 1024          # query rows per core
NK = 2048          # key rows per core (full sequence)
P = 128
NTH = 4            # C tiles of 128
NT = 16            # key tiles of 128
NHT = 16           # hidden tiles of 128
EPS = 1e-6
N_CORES = 8


def _emit(nc, aps, stop_after="full"):
    """Emit the full per-core program under a TileContext."""
    qT, qTb, kT, vaug = aps["qT"], aps["qTb"], aps["kT"], aps["vaug"]
    fc1wT, fc2wT = aps["fc1wT"], aps["fc2wT"]
    fc1b, fc2b = aps["fc1b"], aps["fc2b"]
    ln1w, ln1b, ln2w, ln2b = aps["ln1w"], aps["ln1b"], aps["ln2w"], aps["ln2b"]
    outT = aps["outT"]

    with tile.TileContext(nc) as tc, ExitStack() as ctx:
        singles = ctx.enter_context(tc.tile_pool(name="singles", bufs=1))
        lncast = ctx.enter_context(tc.tile_pool(name="lncast", bufs=4))

        # ---- persistent SBUF tensors -------------------------------------
        qres = [singles.tile([P, NQ], F32, name=f"qres{t}") for t in range(NTH)]
        qb = [singles.tile([P, NQ], BF16, name=f"qb{t}") for t in range(NTH)]
        q2 = [singles.tile([P, NQ], F32, name=f"q2_{t}") for t in range(NTH)]
        q2b = [singles.tile([P, NQ], BF16, name=f"q2b{t}") for t in range(NTH)]
        fc1w = [singles.tile([P, HID], BF16, name=f"fc1w{t}") for t in range(NTH)]
        fc2w = [singles.tile([P, C], BF16, name=f"fc2w{t}") for t in range(NHT)]
        ones_col = singles.tile([P, 1], F32)
        ones_col_b = singles.tile([P, 1], BF16)
        ones_row = singles.tile([1, P], F32)
        fc1b_sb = singles.tile([P, NHT], F32)
        fc2b_sb = singles.tile([P, NTH], F32)
        ln1w_sb = singles.tile([P, NTH], F32)
        ln1b_sb = singles.tile([P, NTH], F32)
        ln2w_sb = singles.tile([P, NTH], F32)
        ln2b_sb = singles.tile([P, NTH], F32)
        mean_bc = singles.tile([P, NQ], F32)         # LN mean broadcast
        rstd_bc = singles.tile([P, NQ], F32)         # LN rstd broadcast
        row0 = singles.tile([1, NQ], F32)            # LN row scratch: mean
        row1 = singles.tile([1, NQ], F32)            # LN row scratch: msq/var/rstd
        row2 = singles.tile([1, NQ], F32)            # LN row scratch: tmp
        eps_sb = singles.tile([1, 1], F32)

        nc.vector.memset(ones_col, 1.0)
        nc.vector.memset(ones_col_b, 1.0 / C)   # stats matmuls fold the 1/C
        nc.vector.memset(ones_row, 1.0)
        nc.vector.memset(eps_sb, EPS)

        # ---- input loads -------------------------------------------------
        nc.sync.dma_start(out=qb[0], in_=qTb[0:P, :])

        vaug_r = vaug.rearrange("(nt p) c -> p nt c", p=P)   # [128, 16, 520]

        # ================= PHASE A: attention ============================
        with tc.tile_pool(name="psA", bufs=2, space="PSUM") as psA, \
             tc.tile_pool(name="kpool", bufs=2) as kpool, \
             tc.tile_pool(name="vpool", bufs=2) as vpool, \
             tc.tile_pool(name="probs", bufs=4) as probs_pool, \
             tc.tile_pool(name="stage", bufs=2) as stage_pool, \
             tc.tile_pool(name="denp", bufs=2) as den_pool:

            ln1_pre = [None] * NTH   # (rb, sq) bf16 casts for LN1, made early

            def emit_scale(po, rec, t, hof):
                # broadcast rec along partitions via a K=1 ones-matmul, then
                # scale the head's attention and accumulate onto the residual
                bcp = psA.tile([D, NQ], F32, tag="sc", name="bcp")
                for j in range(2):
                    nc.tensor.matmul(bcp[:, j * 512:(j + 1) * 512],
                                     lhsT=ones_row[:, 0:D],
                                     rhs=rec[:, j * 512:(j + 1) * 512],
                                     start=True, stop=True)
                bc = stage_pool.tile([D, NQ], F32, tag="bc")
                nc.vector.tensor_copy(out=bc, in_=bcp)
                st = stage_pool.tile([D, NQ], F32, tag="st")
                nc.vector.tensor_tensor(out=st, in0=po[0:D, :], in1=bc, op=ALU.mult)
                nc.gpsimd.dma_start(out=qres[t][hof:hof + D, :], in_=st, accum_op=ALU.add)
                if hof == D:
                    # tile t's residual is complete: emit LN1's bf16 casts now
                    # so they run on the otherwise-idle DVE during attention
                    rb = lncast.tile([P, NQ], BF16, tag="rb", name=f"rb{t}")
                    nc.vector.tensor_copy(out=rb, in_=qres[t])
                    sq = lncast.tile([P, NQ], BF16, tag="sq", name=f"sq{t}")
                    nc.vector.tensor_tensor(out=sq, in0=qres[t], in1=qres[t], op=ALU.mult)
                    ln1_pre[t] = (rb, sq)

            pending = None
            for h in range(H):
                t, hp = h // 2, h % 2
                hof = hp * D
                if hp == 0:
                    kt = kpool.tile([P, NK], BF16, tag="kt")
                    if h == 0:
                        # split head 0's key load so the first QKT only waits
                        # on the first few key tiles
                        nc.sync.dma_start(out=kt[:, 0:512], in_=kT[0:P, 0:512])
                        nc.sync.dma_start(out=kt[:, 512:], in_=kT[0:P, 512:])
                    else:
                        nc.sync.dma_start(out=kt, in_=kT[t * P:(t + 1) * P, :])
                vau = vpool.tile([P, NT, D + 1], BF16, tag="vau")
                nc.sync.dma_start(out=vau, in_=vaug_r[:, :, h * (D + 1):(h + 1) * (D + 1)])
                if h == 0:
                    # remaining loads, queued behind head 0's qb/kt/vau so the
                    # first matmuls start as early as possible
                    for t2_ in range(1, NTH):
                        nc.sync.dma_start(out=qb[t2_], in_=qTb[t2_ * P:(t2_ + 1) * P, :])
                    for t2_ in range(NTH):
                        nc.sync.dma_start(out=qres[t2_], in_=qT[t2_ * P:(t2_ + 1) * P, :])
                    nc.sync.dma_start(out=ln1w_sb, in_=ln1w.rearrange("(t p) -> p t", p=P))
                    nc.sync.dma_start(out=ln1b_sb, in_=ln1b.rearrange("(t p) -> p t", p=P))
                po = psA.tile([D + 1, NQ], F32, tag="pv")
                for nt in range(NT):
                    ps = psA.tile([P, NQ], F32, tag="sc")
                    pr = probs_pool.tile([P, NQ], BF16, tag="pr")
                    for j in range(2):
                        nc.tensor.matmul(
                            ps[:, j * 512:(j + 1) * 512],
                            lhsT=kt[hof:hof + D, nt * P:(nt + 1) * P],
                            rhs=qb[t][hof:hof + D, j * 512:(j + 1) * 512],
                            start=True, stop=True,
                        )
                    nc.scalar.activation(pr, ps, AF.Exp, scale=0.125)
                    for j in range(2):
                        nc.tensor.matmul(
                            po[:, j * 512:(j + 1) * 512],
                            lhsT=vau[:, nt, :],
                            rhs=pr[:, j * 512:(j + 1) * 512],
                            start=(nt == 0), stop=(nt == NT - 1),
                        )
                    if nt == 3 and pending is not None:
                        emit_scale(*pending)
                        pending = None
                # denominators -> reciprocal now; the broadcast matmul + scale
                # are deferred into the NEXT head's nt loop so the PE stream
                # isn't blocked behind them while the DVE reciprocal runs.
                den = den_pool.tile([1, NQ], F32, tag="den")
                rec = den_pool.tile([1, NQ], F32, tag="rec")
                scr = den_pool.tile([1, NQ], F32, tag="scr")
                nc.vector.tensor_copy(out=den, in_=po[D:D + 1, :])
                nc.vector.reciprocal_approx_accurate(out=rec, in_=den, scratch=scr)
                pending = (po, rec, t, hof)
            emit_scale(*pending)

        # deferred weight/bias loads: overlap LN1 compute, stay behind the
        # attention-phase kt/vau transfers in the DMA queues
        for t in range(NTH):
            nc.sync.dma_start(out=fc1w[t], in_=fc1wT[t * P:(t + 1) * P, :])
        for t in range(NHT):
            nc.sync.dma_start(out=fc2w[t], in_=fc2wT[t * P:(t + 1) * P, :])
        nc.sync.dma_start(out=fc1b_sb, in_=fc1b.rearrange("(t p) -> p t", p=P))
        nc.sync.dma_start(out=fc2b_sb, in_=fc2b.rearrange("(t p) -> p t", p=P))
        nc.sync.dma_start(out=ln2w_sb, in_=ln2w.rearrange("(t p) -> p t", p=P))
        nc.sync.dma_start(out=ln2b_sb, in_=ln2b.rearrange("(t p) -> p t", p=P))

        if stop_after == "attn":
            for t in range(NTH):
                nc.sync.dma_start(out=outT[t * P:(t + 1) * P, :], in_=qres[t])
            return

        # ================= LN helper =====================================
        # Stats matmuls use bf16 operands (fp32 matmul is quarter-rate); a
        # 512-term mean/var in bf16 loses <0.05% accuracy.
        def layernorm(src_tiles, dst_tiles, w_sb, b_sb, pool_name, pre=None,
                      apply_wb=True, post_half=None):
            with tc.tile_pool(name=pool_name + "_ps", bufs=1, space="PSUM") as psL, \
                 tc.tile_pool(name=pool_name + "_sq", bufs=2) as sq_pool:
                psum = psL.tile([1, NQ], F32, tag="sum")
                psumsq = psL.tile([1, NQ], F32, tag="sumsq")
                for t in range(NTH):
                    if pre is not None and pre[t] is not None:
                        rb, sq = pre[t]
                    else:
                        rb = sq_pool.tile([P, NQ], BF16, tag="rb")
                        nc.vector.tensor_copy(out=rb, in_=src_tiles[t])
                        sq = sq_pool.tile([P, NQ], BF16, tag="sq")
                        nc.vector.tensor_tensor(out=sq, in0=src_tiles[t], in1=src_tiles[t], op=ALU.mult)
                    for j in range(2):
                        sl = slice(j * 512, (j + 1) * 512)
                        nc.tensor.matmul(psum[:, sl], lhsT=ones_col_b, rhs=rb[:, sl],
                                         start=(t == 0), stop=(t == NTH - 1))
                        nc.tensor.matmul(psumsq[:, sl], lhsT=ones_col_b, rhs=sq[:, sl],
                                         start=(t == 0), stop=(t == NTH - 1))
                # psum/psumsq already hold mean and E[x^2] (1/C in stats lhsT)
                nc.vector.tensor_copy(out=row0, in_=psum)              # mean
                nc.vector.tensor_tensor(out=row2, in0=row0, in1=row0, op=ALU.mult)
                nc.vector.scalar_tensor_tensor(out=row1, in0=psumsq, scalar=0.0,
                                               in1=row2, op0=ALU.add, op1=ALU.subtract)  # var
                # rstd via Sqrt + DVE approx-reciprocal: one ACT table set and
                # no Ln/Exp set thrashing (ln and exp live in different sets)
                nc.scalar.activation(row2, row1, AF.Sqrt, bias=eps_sb[0:1, 0:1])
                nc.vector.reciprocal_approx_fast(out=row1, in_=row2)   # rstd
                for row, dst in ((row0, mean_bc), (row1, rstd_bc)):
                    bcp = psL.tile([P, NQ], F32, tag="bcp")
                    for j in range(2):
                        nc.tensor.matmul(bcp[:, j * 512:(j + 1) * 512],
                                         lhsT=ones_row,
                                         rhs=row[:, j * 512:(j + 1) * 512],
                                         start=True, stop=True)
                    nc.vector.tensor_copy(out=dst, in_=bcp)
                # apply in column halves so downstream consumers of half 0
                # (fc1's jq=0 block) can start while half 1 is still applying
                for jh in range(2):
                    slh = slice(jh * 512, (jh + 1) * 512)
                    for t in range(NTH):
                        nc.vector.tensor_tensor(out=dst_tiles[t][:, slh], in0=src_tiles[t][:, slh],
                                                in1=mean_bc[:, slh], op=ALU.subtract)
                        nc.vector.tensor_tensor(out=dst_tiles[t][:, slh], in0=dst_tiles[t][:, slh],
                                                in1=rstd_bc[:, slh], op=ALU.mult)
                        if apply_wb:
                            nc.vector.tensor_scalar(out=dst_tiles[t][:, slh], in0=dst_tiles[t][:, slh],
                                                    scalar1=w_sb[:, t:t + 1], scalar2=b_sb[:, t:t + 1],
                                                    op0=ALU.mult, op1=ALU.add)
                    if post_half is not None:
                        post_half(jh)

        # ================= PHASE B: LN1 ==================================
        def _q2b_cast_half(jh):
            slh = slice(jh * 512, (jh + 1) * 512)
            for t in range(NTH):
                nc.vector.tensor_copy(out=q2b[t][:, slh], in_=q2[t][:, slh])

        layernorm(qres, q2, ln1w_sb, ln1b_sb, "ln1", pre=ln1_pre,
                  post_half=_q2b_cast_half)

        if stop_after == "ln1":
            for t in range(NTH):
                nc.sync.dma_start(out=outT[t * P:(t + 1) * P, :], in_=q2[t])
            return

        # ================= PHASE C: MLP + PHASE D: LN2 ===================
        # h_T = gelu(fc1_w @ q2_T + b1);  mlp_T = fc2_w @ h_T + b2;  r2 = q2 + mlp
        # r2 overwrites qres.  LN2 runs per nq-half: half 0's whole chain
        # (casts -> stats -> row math -> broadcast -> apply) is interleaved
        # into MLP's jq=1 emission so it overlaps on otherwise-busy engines;
        # only half 1 remains as the tail.  LN2's w/b affine is applied on the
        # host during the gather.  PSUM: 4 (mlp acc) + 2 (fc1) + 2 (ln2) = 8.
        with tc.tile_pool(name="psM", bufs=1, space="PSUM") as psM, \
             tc.tile_pool(name="psF", bufs=2, space="PSUM") as psF, \
             tc.tile_pool(name="ln2p", bufs=2, space="PSUM") as ln2p, \
             tc.tile_pool(name="hpool", bufs=3) as hpool:

            def ln2_cast_half(jh):
                slh = slice(jh * 512, (jh + 1) * 512)
                pre = []
                for t in range(NTH):
                    rbh = lncast.tile([P, 512], BF16, tag="rb2", name=f"rb2_{jh}_{t}")
                    nc.vector.tensor_copy(out=rbh, in_=qres[t][:, slh])
                    sqh = lncast.tile([P, 512], BF16, tag="sq2", name=f"sq2_{jh}_{t}")
                    nc.vector.tensor_tensor(out=sqh, in0=qres[t][:, slh],
                                            in1=qres[t][:, slh], op=ALU.mult)
                    pre.append((rbh, sqh))
                return pre

            def ln2_stats_half(pre, jh):
                s_ps = ln2p.tile([1, 512], F32, tag="s", name=f"sum2_{jh}")
                q_ps = ln2p.tile([1, 512], F32, tag="s", name=f"sumsq2_{jh}")
                for t in range(NTH):
                    rbh, sqh = pre[t]
                    nc.tensor.matmul(s_ps, lhsT=ones_col_b, rhs=rbh,
                                     start=(t == 0), stop=(t == NTH - 1))
                    nc.tensor.matmul(q_ps, lhsT=ones_col_b, rhs=sqh,
                                     start=(t == 0), stop=(t == NTH - 1))
                return s_ps, q_ps

            def ln2_smalls_half(s_ps, q_ps, jh, after=None):
                slh = slice(jh * 512, (jh + 1) * 512)
                r0, r1, r2 = row0[0:1, slh], row1[0:1, slh], row2[0:1, slh]
                nc.vector.tensor_copy(out=r0, in_=s_ps)                     # mean
                nc.vector.tensor_tensor(out=r2, in0=r0, in1=r0, op=ALU.mult)
                nc.vector.scalar_tensor_tensor(out=r1, in0=q_ps, scalar=0.0,
                                               in1=r2, op0=ALU.add, op1=ALU.subtract)
                bias = eps_sb[0:1, 0:1]
                if after is not None:
                    # eps recomputed FROM the given tile (x*0 + eps) purely to
                    # order this Sqrt after it on the ACT queue, so the gelu
                    # table set isn't reloaded mid-stream
                    dep_eps = lncast.tile([1, 1], F32, tag="deps")
                    nc.vector.tensor_scalar(out=dep_eps, in0=after[0:1, 0:1],
                                            scalar1=0.0, scalar2=EPS,
                                            op0=ALU.mult, op1=ALU.add)
                    bias = dep_eps
                nc.scalar.activation(r2, r1, AF.Sqrt, bias=bias)
                nc.vector.reciprocal_approx_fast(out=r1, in_=r2)            # rstd

            def ln2_bcast_half(jh):
                slh = slice(jh * 512, (jh + 1) * 512)
                for row, dst in ((row0, mean_bc), (row1, rstd_bc)):
                    bcp = ln2p.tile([P, 512], F32, tag="s", name=f"bcp2_{jh}")
                    nc.tensor.matmul(bcp, lhsT=ones_row, rhs=row[0:1, slh],
                                     start=True, stop=True)
                    nc.vector.tensor_copy(out=dst[:, slh], in_=bcp)

            def ln2_apply_half(jh):
                slh = slice(jh * 512, (jh + 1) * 512)
                for t in range(NTH):
                    nc.vector.tensor_tensor(out=q2[t][:, slh], in0=qres[t][:, slh],
                                            in1=mean_bc[:, slh], op=ALU.subtract)
                    nc.vector.tensor_tensor(out=q2[t][:, slh], in0=q2[t][:, slh],
                                            in1=rstd_bc[:, slh], op=ALU.mult)

            ln2_pre0 = None
            ln2_st0 = None
            for jq in range(2):
                sl = slice(jq * 512, (jq + 1) * 512)
                pm = [psM.tile([P, 512], F32, tag=f"mlp{co}", name=f"pm{co}") for co in range(NTH)]
                for kk in range(NHT):
                    ph = psF.tile([P, 512], F32, tag="fc1")
                    for c in range(NTH):
                        nc.tensor.matmul(ph, lhsT=fc1w[c][:, kk * P:(kk + 1) * P],
                                         rhs=q2b[c][:, sl], start=(c == 0), stop=(c == NTH - 1))
                    hs = hpool.tile([P, 512], BF16, tag="hs")
                    nc.scalar.activation(hs, ph, AF.Gelu, bias=fc1b_sb[:, kk:kk + 1])
                    for co in range(NTH):
                        nc.tensor.matmul(pm[co], lhsT=fc2w[kk][:, co * P:(co + 1) * P],
                                         rhs=hs, start=(kk == 0), stop=(kk == NHT - 1))
                    if jq == 1:
                        # half-0 LN2 interleave; the Sqrt row op waits until
                        # after the LAST gelu emission so the ACT gelu table
                        # set is never reloaded mid-MLP
                        if kk == 4:
                            ln2_st0 = ln2_stats_half(ln2_pre0, 0)
                        elif kk == NHT - 1:
                            ln2_smalls_half(*ln2_st0, 0, after=hs)
                if jq == 1:
                    ln2_bcast_half(0)
                for co in range(NTH):
                    # r2 = (mlp_psum + fc2_b) + q2 in one fused DVE op
                    nc.vector.scalar_tensor_tensor(
                        out=qres[co][:, sl], in0=pm[co], scalar=fc2b_sb[:, co:co + 1],
                        in1=q2[co][:, sl], op0=ALU.add, op1=ALU.add)
                if jq == 0:
                    ln2_pre0 = ln2_cast_half(0)
                else:
                    ln2_apply_half(0)

            if stop_after == "mlp":
                for t in range(NTH):
                    nc.sync.dma_start(out=outT[t * P:(t + 1) * P, :], in_=qres[t])
                return

            # tail: LN2 half 1 only
            pre1 = ln2_cast_half(1)
            st1 = ln2_stats_half(pre1, 1)
            ln2_smalls_half(*st1, 1)
            ln2_bcast_half(1)
            ln2_apply_half(1)

        for t in range(NTH):
            nc.sync.dma_start(out=outT[t * P:(t + 1) * P, :], in_=q2[t])


def build_program(stop_after="full"):
    nc = bacc.Bacc("TRN2", target_bir_lowering=False, debug=False)
    aps = {}
    for name, shape, dt_ in [
        ("qT", [C, NQ], F32), ("qTb", [C, NQ], BF16),
        ("kT", [C, NK], BF16), ("vaug", [NK, H * (D + 1)], BF16),
        ("fc1wT", [C, HID], BF16), ("fc2wT", [HID, C], BF16),
        ("fc1b", [HID], F32), ("fc2b", [C], F32),
        ("ln1w", [C], F32), ("ln1b", [C], F32), ("ln2w", [C], F32), ("ln2b", [C], F32),
    ]:
        aps[name] = nc.dram_tensor(name, shape, dt_, kind="ExternalInput").ap()
    aps["outT"] = nc.dram_tensor("outT", [C, NQ], F32, kind="ExternalOutput").ap()
    _emit(nc, aps, stop_after=stop_after)
    nc.finalize()
    return nc


def make_in_maps(q, k, v, fc1_w, fc1_b, fc2_w, fc2_b, ln1_w, ln1_b, ln2_w, ln2_b):
    import ml_dtypes
    bf16 = ml_dtypes.bfloat16
    f = lambda x: np.ascontiguousarray(np.asarray(x), dtype=np.float32)
    fb = lambda x: np.ascontiguousarray(np.asarray(x, dtype=np.float32).astype(bf16))
    q, k, v = f(q), f(k), f(v)
    common = dict(
        fc1wT=fb(np.asarray(fc1_w).T), fc2wT=fb(np.asarray(fc2_w).T),
        fc1b=f(fc1_b), fc2b=f(fc2_b),
        ln1w=f(ln1_w), ln1b=f(ln1_b), ln2w=f(ln2_w), ln2b=f(ln2_b),
    )
    in_maps = []
    for core in range(N_CORES):
        b, half = core // 2, core % 2
        qT = f(q[b, half * NQ:(half + 1) * NQ, :].T)
        vh = v[b].reshape(NK, H, D)
        vaug = np.concatenate([vh, np.ones((NK, H, 1), np.float32)], axis=2)
        in_maps.append(dict(qT=qT, qTb=fb(qT), kT=fb(k[b].T),
                            vaug=fb(vaug.reshape(NK, H * (D + 1))), **common))
    return in_maps


_PROGRAM = None


def _run(in_maps):
    global _PROGRAM
    if _PROGRAM is None:
        _PROGRAM = build_program()
    res = run_bass_kernel_spmd(_PROGRAM, in_maps, list(range(N_CORES)))
    out = np.empty((B, NSEQ, C), dtype=np.float32)
    w2 = in_maps[0]["ln2w"][None, :]
    b2 = in_maps[0]["ln2b"][None, :]
    for core in range(N_CORES):
        b, half = core // 2, core % 2
        out[b, half * NQ:(half + 1) * NQ, :] = res.results[core]["outT"].T * w2 + b2
    return out


def _subproc_main(in_path, out_path):
    d = np.load(in_path)
    in_maps = [{k[3:]: d[k] for k in d.files if k.startswith(f"c{core}_")}
               for core in range(N_CORES)]
    np.save(out_path, _run(in_maps))


def _run_subprocess(in_maps):
    """Fresh-process fallback: the device occasionally reports
    NRT_EXEC_UNIT_UNRECOVERABLE on the first execution of a newly loaded NEFF;
    a fresh PJRT client reliably recovers."""
    import subprocess
    import sys
    import tempfile
    import os
    here = os.path.dirname(os.path.abspath(__file__))
    with tempfile.TemporaryDirectory() as td:
        in_path = os.path.join(td, "in.npz")
        out_path = os.path.join(td, "out.npy")
        np.savez(in_path, **{f"c{c}_{k}": v for c, m in enumerate(in_maps)
                             for k, v in m.items()})
        code = (f"import sys; sys.path.insert(0, {here!r}); "
                f"import kernel; kernel._subproc_main({in_path!r}, {out_path!r})")
        last = None
        for _ in range(3):
            try:
                subprocess.run([sys.executable, "-c", code], check=True, timeout=1800)
                return np.load(out_path)
            except Exception as ex:  # noqa: BLE001
                last = ex
        raise last


def kernel(q, k, v, fc1_w, fc1_b, fc2_w, fc2_b, ln1_w, ln1_b, ln2_w, ln2_b):
    in_maps = make_in_maps(q, k, v, fc1_w, fc1_b, fc2_w, fc2_b,
                           ln1_w, ln1_b, ln2_w, ln2_b)
    for _ in range(2):
        try:
            return _run(in_maps)
        except Exception:  # noqa: BLE001 — transient NRT exec faults
            pass
    return _run_subprocess(in_maps)


# revision 63
# speedup vs baseline: 1.0288x; 1.0288x over previous
"""Trainium2 Bass kernel for a dense transformer encoder layer.

Reference computation (B=4, N=2048, C=512, H=8 heads, HIDE=2048):
    attn = MHA(q, k, v)                      # full bidirectional softmax attention
    q2   = LN1(q + attn)
    mlp  = fc2(gelu(fc1(q2)))
    out  = LN2(q2 + mlp)

Sharding: data-parallel over (batch x query-sequence-half): 8 cores, each
handles 1024 query rows of one batch, with the full 2048 keys/values of that
batch resident per core.  Zero collectives.

Everything on-device runs in "transposed land" ([C, nq] layouts, channels on
partitions) so that no on-device transposes are needed anywhere:
  * scores_T[nk, nq] = kT_h.T @ qT_h           (lhsT = kT slice, K = 64)
  * probs_T = exp(scores_T / 8)                (ACT, no max-subtraction; scores
                                                are O(6) so exp is safe in fp32)
  * attn_T[d, nq]   = vaug_h.T @ probs_T       (lhsT = v augmented with a ones
                                                column -> row 64 = softmax denom)
  * LN stats over C via ones-column matmuls (partition-axis reduction on PE)
  * rstd = exp(-0.5 * ln(var + eps))           (ACT Rsqrt is banned for accuracy)
  * MLP stays transposed: h_T = fc1_w @ q2_T, mlp_T = fc2_w @ h_T
The final output is written transposed ([C, nq]) and un-transposed on host.
"""

import numpy as np
from contextlib import ExitStack

import concourse.bass as bass
import concourse.mybir as mybir
import concourse.tile as tile
from concourse import bacc
from concourse.bass_utils import run_bass_kernel_spmd

F32 = mybir.dt.float32
F32R = mybir.dt.float32r
AF = mybir.ActivationFunctionType
ALU = mybir.AluOpType


BF16 = mybir.dt.bfloat16

B = 4
H = 8
C = 512
D = 64
HID = 2048
NSEQ = 2048
NQ = 1024          # query rows per core
NK = 2048          # key rows per core (full sequence)
P = 128
NTH = 4            # C tiles of 128
NT = 16            # key tiles of 128
NHT = 16           # hidden tiles of 128
EPS = 1e-6
N_CORES = 8


def _emit(nc, aps, stop_after="full"):
    """Emit the full per-core program under a TileContext."""
    qT, qTb, kT, vaug = aps["qT"], aps["qTb"], aps["kT"], aps["vaug"]
    fc1wT, fc2wT = aps["fc1wT"], aps["fc2wT"]
    fc1b, fc2b = aps["fc1b"], aps["fc2b"]
    ln1w, ln1b, ln2w, ln2b = aps["ln1w"], aps["ln1b"], aps["ln2w"], aps["ln2b"]
    outT = aps["outT"]

    with tile.TileContext(nc) as tc, ExitStack() as ctx:
        singles = ctx.enter_context(tc.tile_pool(name="singles", bufs=1))
        lncast = ctx.enter_context(tc.tile_pool(name="lncast", bufs=4))

        # ---- persistent SBUF tensors -------------------------------------
        qres = [singles.tile([P, NQ], F32, name=f"qres{t}") for t in range(NTH)]
        qb = [singles.tile([P, NQ], BF16, name=f"qb{t}") for t in range(NTH)]
        q2 = [singles.tile([P, NQ], F32, name=f"q2_{t}") for t in range(NTH)]
        q2b = [singles.tile([P, NQ], BF16, name=f"q2b{t}") for t in range(NTH)]
        fc1w = [singles.tile([P, HID], BF16, name=f"fc1w{t}") for t in range(NTH)]
        fc2w = [singles.tile([P, C], BF16, name=f"fc2w{t}") for t in range(NHT)]
        ones_col = singles.tile([P, 1], F32)
        ones_col_b = singles.tile([P, 1], BF16)
        ones_row = singles.tile([1, P], F32)
        fc1b_sb = singles.tile([P, NHT], F32)
        fc2b_sb = singles.tile([P, NTH], F32)
        ln1w_sb = singles.tile([P, NTH], F32)
        ln1b_sb = singles.tile([P, NTH], F32)
        ln2w_sb = singles.tile([P, NTH], F32)
        ln2b_sb = singles.tile([P, NTH], F32)
        mean_bc = singles.tile([P, NQ], F32)         # LN mean broadcast
        rstd_bc = singles.tile([P, NQ], F32)         # LN rstd broadcast
        row0 = singles.tile([1, NQ], F32)            # LN row scratch: mean
        row1 = singles.tile([1, NQ], F32)            # LN row scratch: msq/var/rstd
        row2 = singles.tile([1, NQ], F32)            # LN row scratch: tmp
        eps_sb = singles.tile([1, 1], F32)

        nc.vector.memset(ones_col, 1.0)
        nc.vector.memset(ones_col_b, 1.0 / C)   # stats matmuls fold the 1/C
        nc.vector.memset(ones_row, 1.0)
        nc.vector.memset(eps_sb, EPS)

        # ---- input loads -------------------------------------------------
        nc.sync.dma_start(out=qb[0], in_=qTb[0:P, :])

        vaug_r = vaug.rearrange("(nt p) c -> p nt c", p=P)   # [128, 16, 520]

        # ================= PHASE A: attention ============================
        with tc.tile_pool(name="psA", bufs=2, space="PSUM") as psA, \
             tc.tile_pool(name="kpool", bufs=2) as kpool, \
             tc.tile_pool(name="vpool", bufs=2) as vpool, \
             tc.tile_pool(name="probs", bufs=4) as probs_pool, \
             tc.tile_pool(name="stage", bufs=2) as stage_pool, \
             tc.tile_pool(name="denp", bufs=2) as den_pool:

            ln1_pre = [None] * NTH   # (rb, sq) bf16 casts for LN1, made early

            def emit_scale(po, rec, t, hof):
                # broadcast rec along partitions via a K=1 ones-matmul, then
                # scale the head's attention and accumulate onto the residual
                bcp = psA.tile([D, NQ], F32, tag="sc", name="bcp")
                for j in range(2):
                    nc.tensor.matmul(bcp[:, j * 512:(j + 1) * 512],
                                     lhsT=ones_row[:, 0:D],
                                     rhs=rec[:, j * 512:(j + 1) * 512],
                                     start=True, stop=True)
                bc = stage_pool.tile([D, NQ], F32, tag="bc")
                nc.vector.tensor_copy(out=bc, in_=bcp)
                st = stage_pool.tile([D, NQ], F32, tag="st")
                nc.vector.tensor_tensor(out=st, in0=po[0:D, :], in1=bc, op=ALU.mult)
                nc.gpsimd.dma_start(out=qres[t][hof:hof + D, :], in_=st, accum_op=ALU.add)
                if hof == D:
                    # tile t's residual is complete: emit LN1's bf16 casts now
                    # so they run on the otherwise-idle DVE during attention
                    rb = lncast.tile([P, NQ], BF16, tag="rb", name=f"rb{t}")
                    nc.vector.tensor_copy(out=rb, in_=qres[t])
                    sq = lncast.tile([P, NQ], BF16, tag="sq", name=f"sq{t}")
                    nc.vector.tensor_tensor(out=sq, in0=qres[t], in1=qres[t], op=ALU.mult)
                    ln1_pre[t] = (rb, sq)

            pending = None
            for h in range(H):
                t, hp = h // 2, h % 2
                hof = hp * D
                if hp == 0:
                    kt = kpool.tile([P, NK], BF16, tag="kt")
                    if h == 0:
                        # split head 0's key load so the first QKT only waits
                        # on the first few key tiles
                        nc.sync.dma_start(out=kt[:, 0:512], in_=kT[0:P, 0:512])
                        nc.sync.dma_start(out=kt[:, 512:], in_=kT[0:P, 512:])
                    else:
                        nc.sync.dma_start(out=kt, in_=kT[t * P:(t + 1) * P, :])
                vau = vpool.tile([P, NT, D + 1], BF16, tag="vau")
                nc.sync.dma_start(out=vau, in_=vaug_r[:, :, h * (D + 1):(h + 1) * (D + 1)])
                if h == 0:
                    # remaining loads, queued behind head 0's qb/kt/vau so the
                    # first matmuls start as early as possible
                    for t2_ in range(1, NTH):
                        nc.sync.dma_start(out=qb[t2_], in_=qTb[t2_ * P:(t2_ + 1) * P, :])
                    for t2_ in range(NTH):
                        nc.sync.dma_start(out=qres[t2_], in_=qT[t2_ * P:(t2_ + 1) * P, :])
                    nc.sync.dma_start(out=ln1w_sb, in_=ln1w.rearrange("(t p) -> p t", p=P))
                    nc.sync.dma_start(out=ln1b_sb, in_=ln1b.rearrange("(t p) -> p t", p=P))
                po = psA.tile([D + 1, NQ], F32, tag="pv")
                for nt in range(NT):
                    ps = psA.tile([P, NQ], F32, tag="sc")
                    pr = probs_pool.tile([P, NQ], BF16, tag="pr")
                    for j in range(2):
                        nc.tensor.matmul(
                            ps[:, j * 512:(j + 1) * 512],
                            lhsT=kt[hof:hof + D, nt * P:(nt + 1) * P],
                            rhs=qb[t][hof:hof + D, j * 512:(j + 1) * 512],
                            start=True, stop=True,
                        )
                    nc.scalar.activation(pr, ps, AF.Exp, scale=0.125)
                    for j in range(2):
                        nc.tensor.matmul(
                            po[:, j * 512:(j + 1) * 512],
                            lhsT=vau[:, nt, :],
                            rhs=pr[:, j * 512:(j + 1) * 512],
                            start=(nt == 0), stop=(nt == NT - 1),
                        )
                    if nt == 3 and pending is not None:
                        emit_scale(*pending)
                        pending = None
                # denominators -> reciprocal now; the broadcast matmul + scale
                # are deferred into the NEXT head's nt loop so the PE stream
                # isn't blocked behind them while the DVE reciprocal runs.
                den = den_pool.tile([1, NQ], F32, tag="den")
                rec = den_pool.tile([1, NQ], F32, tag="rec")
                nc.vector.tensor_copy(out=den, in_=po[D:D + 1, :])
                nc.vector.reciprocal_approx_fast(out=rec, in_=den)
                pending = (po, rec, t, hof)
            emit_scale(*pending)

        # deferred weight/bias loads: overlap LN1 compute, stay behind the
        # attention-phase kt/vau transfers in the DMA queues
        for t in range(NTH):
            nc.sync.dma_start(out=fc1w[t], in_=fc1wT[t * P:(t + 1) * P, :])
        for t in range(NHT):
            nc.sync.dma_start(out=fc2w[t], in_=fc2wT[t * P:(t + 1) * P, :])
        nc.sync.dma_start(out=fc1b_sb, in_=fc1b.rearrange("(t p) -> p t", p=P))
        nc.sync.dma_start(out=fc2b_sb, in_=fc2b.rearrange("(t p) -> p t", p=P))
        nc.sync.dma_start(out=ln2w_sb, in_=ln2w.rearrange("(t p) -> p t", p=P))
        nc.sync.dma_start(out=ln2b_sb, in_=ln2b.rearrange("(t p) -> p t", p=P))

        if stop_after == "attn":
            for t in range(NTH):
                nc.sync.dma_start(out=outT[t * P:(t + 1) * P, :], in_=qres[t])
            return

        # ================= LN helper =====================================
        # Stats matmuls use bf16 operands (fp32 matmul is quarter-rate); a
        # 512-term mean/var in bf16 loses <0.05% accuracy.
        def layernorm(src_tiles, dst_tiles, w_sb, b_sb, pool_name, pre=None,
                      apply_wb=True, post_half=None):
            with tc.tile_pool(name=pool_name + "_ps", bufs=1, space="PSUM") as psL, \
                 tc.tile_pool(name=pool_name + "_sq", bufs=2) as sq_pool:
                psum = psL.tile([1, NQ], F32, tag="sum")
                psumsq = psL.tile([1, NQ], F32, tag="sumsq")
                for t in range(NTH):
                    if pre is not None and pre[t] is not None:
                        rb, sq = pre[t]
                    else:
                        rb = sq_pool.tile([P, NQ], BF16, tag="rb")
                        nc.vector.tensor_copy(out=rb, in_=src_tiles[t])
                        sq = sq_pool.tile([P, NQ], BF16, tag="sq")
                        nc.vector.tensor_tensor(out=sq, in0=src_tiles[t], in1=src_tiles[t], op=ALU.mult)
                    for j in range(2):
                        sl = slice(j * 512, (j + 1) * 512)
                        nc.tensor.matmul(psum[:, sl], lhsT=ones_col_b, rhs=rb[:, sl],
                                         start=(t == 0), stop=(t == NTH - 1))
                        nc.tensor.matmul(psumsq[:, sl], lhsT=ones_col_b, rhs=sq[:, sl],
                                         start=(t == 0), stop=(t == NTH - 1))
                # psum/psumsq already hold mean and E[x^2] (1/C in stats lhsT)
                nc.vector.tensor_copy(out=row0, in_=psum)              # mean
                nc.vector.tensor_tensor(out=row2, in0=row0, in1=row0, op=ALU.mult)
                nc.vector.scalar_tensor_tensor(out=row1, in0=psumsq, scalar=0.0,
                                               in1=row2, op0=ALU.add, op1=ALU.subtract)  # var
                # rstd via Sqrt + DVE approx-reciprocal: one ACT table set and
                # no Ln/Exp set thrashing (ln and exp live in different sets)
                nc.scalar.activation(row2, row1, AF.Sqrt, bias=eps_sb[0:1, 0:1])
                nc.vector.reciprocal_approx_fast(out=row1, in_=row2)   # rstd
                for row, dst in ((row0, mean_bc), (row1, rstd_bc)):
                    bcp = psL.tile([P, NQ], F32, tag="bcp")
                    for j in range(2):
                        nc.tensor.matmul(bcp[:, j * 512:(j + 1) * 512],
                                         lhsT=ones_row,
                                         rhs=row[:, j * 512:(j + 1) * 512],
                                         start=True, stop=True)
                    nc.vector.tensor_copy(out=dst, in_=bcp)
                # apply in column halves so downstream consumers of half 0
                # (fc1's jq=0 block) can start while half 1 is still applying
                for jh in range(2):
                    slh = slice(jh * 512, (jh + 1) * 512)
                    for t in range(NTH):
                        nc.vector.tensor_tensor(out=dst_tiles[t][:, slh], in0=src_tiles[t][:, slh],
                                                in1=mean_bc[:, slh], op=ALU.subtract)
                        nc.vector.tensor_tensor(out=dst_tiles[t][:, slh], in0=dst_tiles[t][:, slh],
                                                in1=rstd_bc[:, slh], op=ALU.mult)
                        if apply_wb:
                            nc.vector.tensor_scalar(out=dst_tiles[t][:, slh], in0=dst_tiles[t][:, slh],
                                                    scalar1=w_sb[:, t:t + 1], scalar2=b_sb[:, t:t + 1],
                                                    op0=ALU.mult, op1=ALU.add)
                    if post_half is not None:
                        post_half(jh)

        # ================= PHASE B: LN1 ==================================
        def _q2b_cast_half(jh):
            slh = slice(jh * 512, (jh + 1) * 512)
            for t in range(NTH):
                nc.vector.tensor_copy(out=q2b[t][:, slh], in_=q2[t][:, slh])

        layernorm(qres, q2, ln1w_sb, ln1b_sb, "ln1", pre=ln1_pre,
                  post_half=_q2b_cast_half)

        if stop_after == "ln1":
            for t in range(NTH):
                nc.sync.dma_start(out=outT[t * P:(t + 1) * P, :], in_=q2[t])
            return

        # ================= PHASE C: MLP + PHASE D: LN2 ===================
        # h_T = gelu(fc1_w @ q2_T + b1);  mlp_T = fc2_w @ h_T + b2;  r2 = q2 + mlp
        # r2 overwrites qres.  LN2 runs per nq-half: half 0's whole chain
        # (casts -> stats -> row math -> broadcast -> apply) is interleaved
        # into MLP's jq=1 emission so it overlaps on otherwise-busy engines;
        # only half 1 remains as the tail.  LN2's w/b affine is applied on the
        # host during the gather.  PSUM: 4 (mlp acc) + 2 (fc1) + 2 (ln2) = 8.
        with tc.tile_pool(name="psM", bufs=1, space="PSUM") as psM, \
             tc.tile_pool(name="psF", bufs=2, space="PSUM") as psF, \
             tc.tile_pool(name="ln2p", bufs=2, space="PSUM") as ln2p, \
             tc.tile_pool(name="hpool", bufs=3) as hpool:

            def ln2_cast_half(jh):
                slh = slice(jh * 512, (jh + 1) * 512)
                pre = []
                for t in range(NTH):
                    rbh = lncast.tile([P, 512], BF16, tag="rb2", name=f"rb2_{jh}_{t}")
                    nc.vector.tensor_copy(out=rbh, in_=qres[t][:, slh])
                    sqh = lncast.tile([P, 512], BF16, tag="sq2", name=f"sq2_{jh}_{t}")
                    nc.vector.tensor_tensor(out=sqh, in0=qres[t][:, slh],
                                            in1=qres[t][:, slh], op=ALU.mult)
                    pre.append((rbh, sqh))
                return pre

            def ln2_stats_half(pre, jh):
                s_ps = ln2p.tile([1, 512], F32, tag="s", name=f"sum2_{jh}")
                q_ps = ln2p.tile([1, 512], F32, tag="s", name=f"sumsq2_{jh}")
                for t in range(NTH):
                    rbh, sqh = pre[t]
                    nc.tensor.matmul(s_ps, lhsT=ones_col_b, rhs=rbh,
                                     start=(t == 0), stop=(t == NTH - 1))
                    nc.tensor.matmul(q_ps, lhsT=ones_col_b, rhs=sqh,
                                     start=(t == 0), stop=(t == NTH - 1))
                return s_ps, q_ps

            def ln2_smalls_half(s_ps, q_ps, jh, after=None):
                slh = slice(jh * 512, (jh + 1) * 512)
                r0, r1, r2 = row0[0:1, slh], row1[0:1, slh], row2[0:1, slh]
                nc.vector.tensor_copy(out=r0, in_=s_ps)                     # mean
                nc.vector.tensor_tensor(out=r2, in0=r0, in1=r0, op=ALU.mult)
                nc.vector.scalar_tensor_tensor(out=r1, in0=q_ps, scalar=0.0,
                                               in1=r2, op0=ALU.add, op1=ALU.subtract)
                bias = eps_sb[0:1, 0:1]
                if after is not None:
                    # eps recomputed FROM the given tile (x*0 + eps) purely to
                    # order this Sqrt after it on the ACT queue, so the gelu
                    # table set isn't reloaded mid-stream
                    dep_eps = lncast.tile([1, 1], F32, tag="deps")
                    nc.vector.tensor_scalar(out=dep_eps, in0=after[0:1, 0:1],
                                            scalar1=0.0, scalar2=EPS,
                                            op0=ALU.mult, op1=ALU.add)
                    bias = dep_eps
                nc.scalar.activation(r2, r1, AF.Sqrt, bias=bias)
                nc.vector.reciprocal_approx_fast(out=r1, in_=r2)            # rstd

            def ln2_bcast_half(jh):
                slh = slice(jh * 512, (jh + 1) * 512)
                for row, dst in ((row0, mean_bc), (row1, rstd_bc)):
                    bcp = ln2p.tile([P, 512], F32, tag="s", name=f"bcp2_{jh}")
                    nc.tensor.matmul(bcp, lhsT=ones_row, rhs=row[0:1, slh],
                                     start=True, stop=True)
                    nc.vector.tensor_copy(out=dst[:, slh], in_=bcp)

            def ln2_apply_half(jh):
                slh = slice(jh * 512, (jh + 1) * 512)
                for t in range(NTH):
                    nc.vector.tensor_tensor(out=q2[t][:, slh], in0=qres[t][:, slh],
                                            in1=mean_bc[:, slh], op=ALU.subtract)
                    nc.vector.tensor_tensor(out=q2[t][:, slh], in0=q2[t][:, slh],
                                            in1=rstd_bc[:, slh], op=ALU.mult)

            ln2_pre0 = None
            ln2_st0 = None
            for jq in range(2):
                sl = slice(jq * 512, (jq + 1) * 512)
                pm = [psM.tile([P, 512], F32, tag=f"mlp{co}", name=f"pm{co}") for co in range(NTH)]
                for kk in range(NHT):
                    ph = psF.tile([P, 512], F32, tag="fc1")
                    for c in range(NTH):
                        nc.tensor.matmul(ph, lhsT=fc1w[c][:, kk * P:(kk + 1) * P],
                                         rhs=q2b[c][:, sl], start=(c == 0), stop=(c == NTH - 1))
                    hs = hpool.tile([P, 512], BF16, tag="hs")
                    nc.scalar.activation(hs, ph, AF.Gelu, bias=fc1b_sb[:, kk:kk + 1])
                    for co in range(NTH):
                        nc.tensor.matmul(pm[co], lhsT=fc2w[kk][:, co * P:(co + 1) * P],
                                         rhs=hs, start=(kk == 0), stop=(kk == NHT - 1))
                    if jq == 1:
                        # half-0 LN2 interleave; the Sqrt row op waits until
                        # after the LAST gelu emission so the ACT gelu table
                        # set is never reloaded mid-MLP
                        if kk == 4:
                            ln2_st0 = ln2_stats_half(ln2_pre0, 0)
                        elif kk == NHT - 1:
                            ln2_smalls_half(*ln2_st0, 0, after=hs)
                if jq == 1:
                    ln2_bcast_half(0)
                for co in range(NTH):
                    # r2 = (mlp_psum + fc2_b) + q2 in one fused DVE op
                    nc.vector.scalar_tensor_tensor(
                        out=qres[co][:, sl], in0=pm[co], scalar=fc2b_sb[:, co:co + 1],
                        in1=q2[co][:, sl], op0=ALU.add, op1=ALU.add)
                if jq == 0:
                    ln2_pre0 = ln2_cast_half(0)
                else:
                    ln2_apply_half(0)

            if stop_after == "mlp":
                for t in range(NTH):
                    nc.sync.dma_start(out=outT[t * P:(t + 1) * P, :], in_=qres[t])
                return

            # tail: LN2 half 1 only
            pre1 = ln2_cast_half(1)
            st1 = ln2_stats_half(pre1, 1)
            ln2_smalls_half(*st1, 1)
            ln2_bcast_half(1)
            ln2_apply_half(1)

        for t in range(NTH):
            nc.sync.dma_start(out=outT[t * P:(t + 1) * P, :], in_=q2[t])


def build_program(stop_after="full"):
    nc = bacc.Bacc("TRN2", target_bir_lowering=False, debug=False)
    aps = {}
    for name, shape, dt_ in [
        ("qT", [C, NQ], F32), ("qTb", [C, NQ], BF16),
        ("kT", [C, NK], BF16), ("vaug", [NK, H * (D + 1)], BF16),
        ("fc1wT", [C, HID], BF16), ("fc2wT", [HID, C], BF16),
        ("fc1b", [HID], F32), ("fc2b", [C], F32),
        ("ln1w", [C], F32), ("ln1b", [C], F32), ("ln2w", [C], F32), ("ln2b", [C], F32),
    ]:
        aps[name] = nc.dram_tensor(name, shape, dt_, kind="ExternalInput").ap()
    aps["outT"] = nc.dram_tensor("outT", [C, NQ], F32, kind="ExternalOutput").ap()
    _emit(nc, aps, stop_after=stop_after)
    nc.finalize()
    return nc


def make_in_maps(q, k, v, fc1_w, fc1_b, fc2_w, fc2_b, ln1_w, ln1_b, ln2_w, ln2_b):
    import ml_dtypes
    bf16 = ml_dtypes.bfloat16
    f = lambda x: np.ascontiguousarray(np.asarray(x), dtype=np.float32)
    fb = lambda x: np.ascontiguousarray(np.asarray(x, dtype=np.float32).astype(bf16))
    q, k, v = f(q), f(k), f(v)
    common = dict(
        fc1wT=fb(np.asarray(fc1_w).T), fc2wT=fb(np.asarray(fc2_w).T),
        fc1b=f(fc1_b), fc2b=f(fc2_b),
        ln1w=f(ln1_w), ln1b=f(ln1_b), ln2w=f(ln2_w), ln2b=f(ln2_b),
    )
    in_maps = []
    for core in range(N_CORES):
        b, half = core // 2, core % 2
        qT = f(q[b, half * NQ:(half + 1) * NQ, :].T)
        vh = v[b].reshape(NK, H, D)
        vaug = np.concatenate([vh, np.ones((NK, H, 1), np.float32)], axis=2)
        in_maps.append(dict(qT=qT, qTb=fb(qT), kT=fb(k[b].T),
                            vaug=fb(vaug.reshape(NK, H * (D + 1))), **common))
    return in_maps


_PROGRAM = None


def _run(in_maps):
    global _PROGRAM
    if _PROGRAM is None:
        _PROGRAM = build_program()
    res = run_bass_kernel_spmd(_PROGRAM, in_maps, list(range(N_CORES)))
    out = np.empty((B, NSEQ, C), dtype=np.float32)
    w2 = in_maps[0]["ln2w"][None, :]
    b2 = in_maps[0]["ln2b"][None, :]
    for core in range(N_CORES):
        b, half = core // 2, core % 2
        out[b, half * NQ:(half + 1) * NQ, :] = res.results[core]["outT"].T * w2 + b2
    return out


def _subproc_main(in_path, out_path):
    d = np.load(in_path)
    in_maps = [{k[3:]: d[k] for k in d.files if k.startswith(f"c{core}_")}
               for core in range(N_CORES)]
    np.save(out_path, _run(in_maps))


def _run_subprocess(in_maps):
    """Fresh-process fallback: the device occasionally reports
    NRT_EXEC_UNIT_UNRECOVERABLE on the first execution of a newly loaded NEFF;
    a fresh PJRT client reliably recovers."""
    import subprocess
    import sys
    import tempfile
    import os
    here = os.path.dirname(os.path.abspath(__file__))
    with tempfile.TemporaryDirectory() as td:
        in_path = os.path.join(td, "in.npz")
        out_path = os.path.join(td, "out.npy")
        np.savez(in_path, **{f"c{c}_{k}": v for c, m in enumerate(in_maps)
                             for k, v in m.items()})
        code = (f"import sys; sys.path.insert(0, {here!r}); "
                f"import kernel; kernel._subproc_main({in_path!r}, {out_path!r})")
        last = None
        for _ in range(3):
            try:
                subprocess.run([sys.executable, "-c", code], check=True, timeout=1800)
                return np.load(out_path)
            except Exception as ex:  # noqa: BLE001
                last = ex
        raise last


def kernel(q, k, v, fc1_w, fc1_b, fc2_w, fc2_b, ln1_w, ln1_b, ln2_w, ln2_b):
    in_maps = make_in_maps(q, k, v, fc1_w, fc1_b, fc2_w, fc2_b,
                           ln1_w, ln1_b, ln2_w, ln2_b)
    for _ in range(2):
        try:
            return _run(in_maps)
        except Exception:  # noqa: BLE001 — transient NRT exec faults
            pass
    return _run_subprocess(in_maps)
